# revision 1
# baseline (speedup 1.0000x reference)
"""Trainium2 Bass kernel for nn_DynamicFiltering.

Computation (per batch b):
  xf = frames of x                     (t, c, h, w)
  y  = LeakyReLU(conv2d(xf, w1, b1), 0.2)
  ker = conv2d(y, w2, b2)              (t, 9, h, w)
  ker = ker - mean_k(ker) + 1/45       (per-pixel kernel over K = t*3*3 = 45)
  out[c,h,w] = sum_{t,k1,k2} x_edge[c,t,h+k1-1,w+k2-1] * ker[t,k1,k2][h,w]

Sharding: 8 cores = 2 batches x 4 H-slabs of 32 rows. Each core gets
pre-padded slabs (host bakes zero padding for convs, edge padding for the
filter patches) so the device program is uniform across cores.

Per-core device program:
  - conv1/conv2 as 9 shifted-offset matmuls accumulating in PSUM (fp32r)
  - LeakyReLU as y0 + (2/3)|y0| with the 0.6 scale folded into w2 host-side
  - per-frame PE transposes bring ker into pixel-partition layout kt
  - kernel normalization + W-edge folds on DVE
  - dynamic filtering with scalar_tensor_tensor (per-partition scalar =
    per-pixel kernel value); the dj column shift is handled by three
    output accumulators plus partition-shifted kt copies (made by DMA,
    which is exempt from the engine start-partition restriction)
  - outputs transposed back via PE; the dj shift collapses to free-dim
    offsets during the merge; DMA out
"""

import numpy as np

DIM = 64
T = 5
H = 128
W = 128
SLAB = 32          # output rows per core
NCORES = 8
GH = 36            # conv grid rows: slab + 2*2 halo
GW = 130           # conv grid cols: W + 2
FR = 34            # filter rows: slab + 2 halo

_PROGRAM_CACHE = {}


def _build_program():
    import concourse.bacc as bacc
    import concourse.mybir as mybir
    from concourse.tile import TileContext

    f32 = mybir.dt.float32
    f32r = mybir.dt.float32r
    Act = mybir.ActivationFunctionType
    Alu = mybir.AluOpType

    nc = bacc.Bacc("TRN2", debug=False)

    xc_d = nc.dram_tensor("xc", [DIM, T, GH, GW], f32r, kind="ExternalInput").ap()
    xt_d = nc.dram_tensor("xt", [W, T, FR, DIM], f32, kind="ExternalInput").ap()
    w1t_d = nc.dram_tensor("w1t", [DIM, 9, DIM], f32r, kind="ExternalInput").ap()
    w2t_d = nc.dram_tensor("w2t", [DIM, 9, 9], f32r, kind="ExternalInput").ap()
    b1_d = nc.dram_tensor("b1c", [DIM, 1], f32, kind="ExternalInput").ap()
    b1s_d = nc.dram_tensor("b1s", [DIM, 1], f32, kind="ExternalInput").ap()
    b2_d = nc.dram_tensor("b2c", [9, 1], f32, kind="ExternalInput").ap()
    ym_d = nc.dram_tensor("ymask", [DIM, 2], f32, kind="ExternalInput").ap()
    em_d = nc.dram_tensor("emask", [W, 1], f32, kind="ExternalInput").ap()
    ef_d = nc.dram_tensor("efold", [W, 1], f32, kind="ExternalInput").ap()
    ea_d = nc.dram_tensor("emA", [W, 1], f32, kind="ExternalInput").ap()
    eb_d = nc.dram_tensor("emB", [W, 1], f32, kind="ExternalInput").ap()
    id_d = nc.dram_tensor("ident", [128, 128], f32, kind="ExternalInput").ap()
    out_d = nc.dram_tensor("out", [DIM, SLAB, W], f32, kind="ExternalOutput").ap()

    with TileContext(nc) as tc:
        with (
            tc.tile_pool(name="consts", bufs=1) as cpool,
            tc.tile_pool(name="xcp", bufs=2) as xcp,
            tc.tile_pool(name="yp", bufs=2) as yp,
            tc.tile_pool(name="stage", bufs=2) as stp,
            tc.tile_pool(name="kerp", bufs=1) as kerp,
            tc.tile_pool(name="ktp", bufs=1) as ktp,
            tc.tile_pool(name="accp", bufs=1) as accp,
            tc.tile_pool(name="obp", bufs=3) as obp,
        ):
            w1t_sb = cpool.tile([DIM, 9, DIM], f32r)
            nc.sync.dma_start(out=w1t_sb, in_=w1t_d)
            w2t_sb = cpool.tile([DIM, 9, 9], f32r)
            nc.sync.dma_start(out=w2t_sb, in_=w2t_d)
            b1_sb = cpool.tile([DIM, 1], f32)
            nc.sync.dma_start(out=b1_sb, in_=b1_d)
            b1s_sb = cpool.tile([DIM, 1], f32)
            nc.sync.dma_start(out=b1s_sb, in_=b1s_d)
            b2_sb = cpool.tile([9, 1], f32)
            nc.sync.dma_start(out=b2_sb, in_=b2_d)
            ym_sb = cpool.tile([DIM, 2], f32)
            nc.sync.dma_start(out=ym_sb, in_=ym_d)
            em_sb = cpool.tile([W, 1], f32)
            nc.sync.dma_start(out=em_sb, in_=em_d)
            id_sb = cpool.tile([128, 128], f32)
            nc.sync.dma_start(out=id_sb, in_=id_d)
            ef_sb = cpool.tile([W, 1], f32)
            nc.sync.dma_start(out=ef_sb, in_=ef_d)
            ea_sb = cpool.tile([W, 1], f32)
            nc.sync.dma_start(out=ea_sb, in_=ea_d)
            eb_sb = cpool.tile([W, 1], f32)
            nc.sync.dma_start(out=eb_sb, in_=eb_d)

            kt = ktp.tile([W, T, SLAB, 9], f32)
            ktr = kt.rearrange("p t r (di dj) -> p t r di dj", di=3, dj=3)
            # partition-shifted kernel copies (DMA is exempt from the engine
            # start-partition restriction): kt_p1[q] = kt[q+1], kt_m1[q] = kt[q-1]
            kt_p1 = ktp.tile([W, T, SLAB, 9], f32)
            kt_m1 = ktp.tile([W, T, SLAB, 9], f32)
            nc.vector.memset(kt_p1[96:128], 0.0)
            nc.vector.memset(kt_m1[0:32], 0.0)
            # three dj-separated accumulators:
            #   acc_dj[q, r, c] += xt[q, t, r+di, c] * m_(t,di,dj)[q - dj + 1, r]
            accs = []
            for dj in range(3):
                a = accp.tile([W, SLAB, DIM], f32, name=f"acc{dj}")
                nc.vector.memset(a, 0.0)
                accs.append(a)
            ksrc = [kt_p1, kt, kt_m1]
            u_sb = accp.tile([W, FR, DIM], f32)

            with (
                tc.tile_pool(name="ps1", bufs=3, space="PSUM") as ps1p,
                tc.tile_pool(name="ps2", bufs=3, space="PSUM") as ps2p,
                tc.tile_pool(name="pst", bufs=2, space="PSUM") as pstp,
            ):
                for f in range(T):
                    xt_f = xcp.tile([W, FR, DIM], f32, tag="xt")
                    nc.sync.dma_start(out=xt_f, in_=xt_d[:, f])
                    if f == 0:
                        nc.gpsimd.tensor_copy(u_sb, xt_f)
                    else:
                        nc.gpsimd.tensor_tensor(u_sb, u_sb, xt_f, Alu.add)
                    xc_f = xcp.tile([DIM, GH, GW], f32r, tag="xc")
                    nc.sync.dma_start(out=xc_f, in_=xc_d[:, f])
                    y_f = yp.tile([DIM, GH, GW], f32r, tag="y")
                    # zero-pad columns read by conv2 (memset can't take f32r)
                    u32 = mybir.dt.uint32
                    nc.gpsimd.memset(y_f[:, 1:35, 0:1].bitcast(u32), 0)
                    nc.gpsimd.memset(y_f[:, 1:35, 129:130].bitcast(u32), 0)

                    # conv1 + leaky relu (scaled by 0.6; compensated in w2t)
                    for rc in range(9):
                        g0 = 1 + 4 * rc
                        nr = 4 if rc < 8 else 2
                        ps = ps1p.tile([DIM, 4, W], f32, tag="ps1")
                        for idx in range(9):
                            di, dj = divmod(idx, 3)
                            rhs = xc_f[:, g0 + di - 1:g0 + di - 1 + nr, dj:dj + W]
                            nc.tensor.matmul(
                                ps[:, :nr, :],
                                lhsT=w1t_sb[:, idx, :],
                                rhs=rhs,
                                start=(idx == 0),
                                stop=(idx == 8),
                            )
                        y0 = stp.tile([DIM, 4, W], f32, tag="y0")
                        a0 = stp.tile([DIM, 4, W], f32, tag="a0")
                        nc.scalar.activation(y0[:, :nr], ps[:, :nr], Act.Identity,
                                             bias=b1_sb, scale=1.0)
                        # a0 = (2/3)|y0| via Abs((2/3) ps + (2/3) b1)
                        nc.scalar.activation(a0[:, :nr], ps[:, :nr], Act.Abs,
                                             bias=b1s_sb, scale=2.0 / 3.0)
                        # y_f = y0 + (2/3)|y0|  == (0.6*y0 + 0.4*|y0|) / 0.6
                        nc.gpsimd.tensor_tensor(
                            y_f[:, g0:g0 + nr, 1:129],
                            y0[:, :nr], a0[:, :nr], Alu.add)

                    # conv2 zero-pads rows outside the image: kill y halo rows
                    # that fall outside (mask is 0 there for edge slabs)
                    nc.scalar.activation(y_f[:, 1:2, 1:129], y_f[:, 1:2, 1:129],
                                          Act.Copy, scale=ym_sb[:, 0:1])
                    nc.scalar.activation(y_f[:, 34:35, 1:129], y_f[:, 34:35, 1:129],
                                          Act.Copy, scale=ym_sb[:, 1:2])

                    # conv2 -> ker_f (9, slab, W); grid row = 2 + r
                    ker_f = kerp.tile([9, SLAB, W], f32, tag="kerf")
                    for rc in range(8):
                        g0 = 2 + 4 * rc
                        ps2 = ps2p.tile([9, 4, W], f32, tag="ps2")
                        for idx in range(9):
                            di, dj = divmod(idx, 3)
                            rhs = y_f[:, g0 + di - 1:g0 + di + 3, dj:dj + W]
                            nc.tensor.matmul(
                                ps2,
                                lhsT=w2t_sb[:, idx, :],
                                rhs=rhs,
                                start=(idx == 0),
                                stop=(idx == 8),
                            )
                        nc.scalar.activation(ker_f[:, 4 * rc:4 * rc + 4, :],
                                             ps2, Act.Identity, bias=b2_sb, scale=1.0)

                    # transpose ker_f (9, r, pc) -> kt[pc, r, 9f..9f+9]
                    for r in range(SLAB):
                        pst = pstp.tile([W, 9], f32, tag="pst")
                        nc.tensor.transpose(pst, ker_f[:, r, :], id_sb[:9, :9])
                        nc.scalar.copy(kt[:, f, r, :], pst)

                    # fold W-edge replicate-pad terms into the dj=1 slot
                    # (raw kernel; the mean correction below compensates):
                    #   pc=0:   m[dj=1] += m[dj=0]   (x col -1 == col 0)
                    #   pc=127: m[dj=1] += m[dj=2]   (x col 128 == col 127)
                    nc.vector.tensor_tensor(ktr[0:1, f, :, :, 1],
                                            ktr[0:1, f, :, :, 1],
                                            ktr[0:1, f, :, :, 0], Alu.add)
                    nc.vector.scalar_tensor_tensor(
                        out=ktr[96:128, f, :, :, 1],
                        in0=ktr[96:128, f, :, :, 2], scalar=em_sb[96:128, :],
                        in1=ktr[96:128, f, :, :, 1], op0=Alu.mult, op1=Alu.add)

                    # shifted copies of this frame's kernel columns
                    nc.sync.dma_start(out=kt_p1[0:127, f], in_=kt[1:128, f])
                    nc.sync.dma_start(out=kt_m1[1:128, f], in_=kt[0:127, f])

                    # un-normalized filtering for this frame (normalization is
                    # unfolded into the c*S term after the loop)
                    for di in range(3):
                        for dj in range(3):
                            kb = ksrc[dj][:, f, :, 3 * di + dj].unsqueeze(2)\
                                .broadcast_to((W, SLAB, DIM))
                            prod = stp.tile([W, SLAB, DIM], f32, tag="prod")
                            nc.vector.tensor_tensor(
                                prod, xt_f[:, di:di + SLAB, :], kb, Alu.mult)
                            nc.vector.tensor_tensor(accs[dj], accs[dj], prod,
                                                    Alu.add)

            # normalization term: out += c * (sum of all 45 patches), with
            # c = 1/45 - mean(ker).  sum45 reads the folded kernel, so undo
            # the fold's double-count at the edge partitions.
            sum45 = ktp.tile([W, SLAB], f32)
            kt_rtn = kt.rearrange("p t r n -> p r t n")
            nc.vector.tensor_reduce(sum45, kt_rtn, axis=mybir.AxisListType.XY,
                                    op=Alu.add)
            c_sb = ktp.tile([W, SLAB], f32)
            nc.vector.tensor_scalar(c_sb, sum45, -1.0 / 45.0, 1.0 / 45.0,
                                    Alu.mult, Alu.add)
            corr = ktp.tile([W, SLAB], f32)
            ktr_r = kt.rearrange("p t r (di dj) -> p r t di dj", di=3, dj=3)
            nc.vector.tensor_reduce(corr[0:32], ktr_r[0:32, :, :, :, 0],
                                    axis=mybir.AxisListType.XY, op=Alu.add)
            nc.vector.tensor_reduce(corr[96:128], ktr_r[96:128, :, :, :, 2],
                                    axis=mybir.AxisListType.XY, op=Alu.add)
            nc.vector.scalar_tensor_tensor(out=c_sb[0:32], in0=corr[0:32],
                                           scalar=ea_sb[0:32], in1=c_sb[0:32],
                                           op0=Alu.mult, op1=Alu.add)
            nc.vector.scalar_tensor_tensor(out=c_sb[96:128], in0=corr[96:128],
                                           scalar=eb_sb[96:128], in1=c_sb[96:128],
                                           op0=Alu.mult, op1=Alu.add)

            # S = 3-row vertical box of U (edge rows already clamped in xt)
            s_sb = accp.tile([W, SLAB, DIM], f32)
            nc.vector.tensor_tensor(s_sb, u_sb[:, 0:SLAB, :],
                                    u_sb[:, 1:SLAB + 1, :], Alu.add)
            nc.vector.tensor_tensor(s_sb, s_sb, u_sb[:, 2:SLAB + 2, :], Alu.add)

            # shifted + edge-doubled variants of c
            c_p1 = ktp.tile([W, SLAB], f32)
            c_m1 = ktp.tile([W, SLAB], f32)
            nc.vector.memset(c_p1[96:128], 0.0)
            nc.vector.memset(c_m1[0:32], 0.0)
            nc.sync.dma_start(out=c_p1[0:127], in_=c_sb[1:128])
            nc.sync.dma_start(out=c_m1[1:128], in_=c_sb[0:127])
            c_c = ktp.tile([W, SLAB], f32)
            nc.vector.tensor_scalar(c_c, c_sb, ef_sb, None, Alu.mult)
            for dj, csrc in ((0, c_p1), (1, c_c), (2, c_m1)):
                cb = csrc.unsqueeze(2).broadcast_to((W, SLAB, DIM))
                prod = stp.tile([W, SLAB, DIM], f32, tag="prod")
                nc.vector.tensor_tensor(prod, s_sb, cb, Alu.mult)
                nc.vector.tensor_tensor(accs[dj], accs[dj], prod, Alu.add)

            # transpose accs (q, r, c) -> (r c, q) chunks; the dj shift is a
            # free-dim offset after transposition:
            #   out[m, pc] = T(acc1)[m, pc] + T(acc0)[m, pc-1] + T(acc2)[m, pc+1]
            a0f = accs[0].rearrange("p r c -> p (r c)")
            a1f = accs[1].rearrange("p r c -> p (r c)")
            a2f = accs[2].rearrange("p r c -> p (r c)")
            out_rcw = out_d.rearrange("c r w -> r c w")
            with tc.tile_pool(name="pso", bufs=2, space="PSUM") as psop:
                for oc in range(16):
                    sl = slice(128 * oc, 128 * (oc + 1))
                    p0 = psop.tile([128, 128], f32, tag="pso0")
                    p1 = psop.tile([128, 128], f32, tag="pso1")
                    p2 = psop.tile([128, 128], f32, tag="pso2")
                    nc.tensor.transpose(p0, a0f[:, sl], id_sb)
                    nc.tensor.transpose(p1, a1f[:, sl], id_sb)
                    nc.tensor.transpose(p2, a2f[:, sl], id_sb)
                    ob = obp.tile([128, 128], f32, tag="ob")
                    nc.vector.tensor_copy(ob, p1)
                    nc.vector.tensor_tensor(ob[:, 1:128], ob[:, 1:128],
                                            p0[:, 0:127], Alu.add)
                    nc.vector.tensor_tensor(ob[:, 0:127], ob[:, 0:127],
                                            p2[:, 1:128], Alu.add)
                    nc.sync.dma_start(out=out_rcw[2 * oc:2 * oc + 2], in_=ob)

    return nc


def _get_program():
    if "nc" not in _PROGRAM_CACHE:
        nc = _build_program()
        nc.finalize()
        _PROGRAM_CACHE["nc"] = nc
    return _PROGRAM_CACHE["nc"]


def _host_prep(x, w1, b1, w2, b2):
    """Build the 8 per-core input maps from full inputs."""
    x = np.asarray(x, dtype=np.float32)
    w1 = np.asarray(w1, dtype=np.float32)
    b1 = np.asarray(b1, dtype=np.float32)
    w2 = np.asarray(w2, dtype=np.float32)
    b2 = np.asarray(b2, dtype=np.float32)

    # w1t[ci, 3*di+dj, o] = w1[o, ci, di, dj]
    w1t = np.ascontiguousarray(w1.transpose(1, 2, 3, 0).reshape(DIM, 9, DIM))
    # w2t[ci, 3*di+dj, o] = 0.6 * w2[o, ci, di, dj]   (leaky-relu scale fold)
    w2t = np.ascontiguousarray(0.6 * w2.transpose(1, 2, 3, 0).reshape(DIM, 9, 9))
    b1c = np.ascontiguousarray(b1.reshape(DIM, 1))
    b1s = np.ascontiguousarray((2.0 / 3.0) * b1.reshape(DIM, 1))
    b2c = np.ascontiguousarray(b2.reshape(9, 1))
    ident = np.eye(128, dtype=np.float32)
    emask = np.zeros((W, 1), dtype=np.float32)
    emask[127, 0] = 1.0
    efold = np.ones((W, 1), dtype=np.float32)
    efold[0, 0] = 2.0
    efold[127, 0] = 2.0
    emA = np.zeros((W, 1), dtype=np.float32)
    emA[0, 0] = 1.0 / 45.0
    emB = np.zeros((W, 1), dtype=np.float32)
    emB[127, 0] = 1.0 / 45.0

    in_maps = []
    for core in range(NCORES):
        b, s = divmod(core, 4)
        r0 = s * SLAB
        # conv input: rows r0-2 .. r0+33 zero padded, cols -1..128 zero padded
        xc = np.zeros((DIM, T, GH, GW), dtype=np.float32)
        lo = max(0, r0 - 2)
        hi = min(H, r0 + 34)
        xc[:, :, lo - (r0 - 2):hi - (r0 - 2), 1:129] = x[b, :, :, lo:hi, :]
        # filter input, pixel-partition: xt[pc, t, r, c] = x[b, c, t, clip(r0-1+r), pc]
        rows = np.clip(np.arange(r0 - 1, r0 + 33), 0, H - 1)
        # x[b][:, :, rows, :] has shape (c, t, 34, w); -> (w, t, 34, c)
        xt = np.ascontiguousarray(x[b][:, :, rows, :].transpose(3, 1, 2, 0))
        # conv2 zero-pad mask for the y halo rows (grid rows 1 and 34)
        ymask = np.ones((DIM, 2), dtype=np.float32)
        if s == 0:
            ymask[:, 0] = 0.0
        if s == 3:
            ymask[:, 1] = 0.0
        in_maps.append({
            "xc": xc, "xt": xt, "w1t": w1t, "w2t": w2t,
            "b1c": b1c, "b1s": b1s, "b2c": b2c, "ymask": ymask, "emask": emask,
            "efold": efold, "emA": emA, "emB": emB, "ident": ident,
        })
    return in_maps


def kernel(x, w1, b1, w2, b2):
    from concourse.bass_utils import run_bass_kernel_spmd

    nc = _get_program()
    in_maps = _host_prep(x, w1, b1, w2, b2)
    res = run_bass_kernel_spmd(nc, in_maps, list(range(NCORES)))
    out = np.zeros((2, DIM, H, W), dtype=np.float32)
    for core in range(NCORES):
        b, s = divmod(core, 4)
        out[b, :, s * SLAB:(s + 1) * SLAB, :] = res.results[core]["out"]
    return out



# revision 4
# speedup vs baseline: 1.6786x; 1.6786x over previous
"""Trainium2 Bass kernel for nn_DynamicFiltering (v2).

Computation (per batch b):
  y  = LeakyReLU(conv2d(x_f, w1, b1), 0.2)      per frame f
  ker = conv2d(y, w2, b2)                        (9, h, w) per frame
  ker = ker - mean_k(ker) + 1/45                 (K = t*3*3 = 45 per pixel)
  out[c,h,w] = sum_{t,k1,k2} x_edge[c,t,h+k1-1,w+k2-1] * ker[t,k1,k2][h,w]

Sharding: 8 cores = 2 batches x 4 H-slabs of 32 rows.

v2 design (vs the fp32r baseline):
  - fp16 everywhere on-chip (gate is 2e-2; measured headroom ~100x)
  - conv contraction packed to 128: x/y stacked with a row-shifted copy in
    partitions 64..127, so the 9-tap conv runs as 6 matmuls (3 pair + 3
    leftover) per 4-row chunk instead of 9
  - LeakyReLU via the hardware Prelu activation (alpha=0.2): one scalar
    instruction per chunk instead of Identity+Abs+gpsimd-add
  - ker (9,r,w) -> (w,r,9) pivot via the XBAR dma_start_transpose (one DMA
    per frame) instead of 160 per-row PE transposes
  - filtering layout [pix, c, r] (r contiguous) so the kernel-broadcast
    tensor_tensor ops hit the DVE 2x 16-bit mode; a couple of taps per
    frame run on gpsimd to share the load
  - final merge: partition-shift the dj accumulators by DMA, add, then one
    XBAR transpose of the merged [w, (c r)] accumulator; out is fp16 and
    the host converts
"""

import numpy as np

DIM = 64
T = 5
H = 128
W = 128
SLAB = 32          # output rows per core
NCORES = 8
GH = 36            # conv grid rows: slab + 2*2 halo
GW = 130           # conv grid cols: W + 2
FR = 34            # filter rows: slab + 2 halo

# taps (di, dj) run on gpsimd instead of DVE (dj=1: no shifted-kt dep)
GP_TAPS = ((0, 1), (2, 1))

_PROGRAM_CACHE = {}


def _build_program():
    import concourse.bacc as bacc
    import concourse.mybir as mybir
    from concourse.tile import TileContext

    f32 = mybir.dt.float32
    f16 = mybir.dt.float16
    Act = mybir.ActivationFunctionType
    Alu = mybir.AluOpType

    nc = bacc.Bacc("TRN2", debug=False)

    xs_d = nc.dram_tensor("xs", [128, T, GH, GW], f16, kind="ExternalInput").ap()
    xt_d = nc.dram_tensor("xt", [W, T, DIM, FR], f16, kind="ExternalInput").ap()
    w1s_d = nc.dram_tensor("w1s", [128, 3, DIM], f16, kind="ExternalInput").ap()
    w1r_d = nc.dram_tensor("w1r", [128, 3, DIM], f16, kind="ExternalInput").ap()
    w2s_d = nc.dram_tensor("w2s", [128, 3, 9], f16, kind="ExternalInput").ap()
    w2r_d = nc.dram_tensor("w2r", [128, 3, 9], f16, kind="ExternalInput").ap()
    b1_d = nc.dram_tensor("b1c", [DIM, 1], f32, kind="ExternalInput").ap()
    b2_d = nc.dram_tensor("b2c", [9, 1], f32, kind="ExternalInput").ap()
    ym_d = nc.dram_tensor("ymask", [DIM, 2], f32, kind="ExternalInput").ap()
    em_d = nc.dram_tensor("emask", [W, 1], f32, kind="ExternalInput").ap()
    ef_d = nc.dram_tensor("efold", [W, 1], f32, kind="ExternalInput").ap()
    ea_d = nc.dram_tensor("emA", [W, 1], f32, kind="ExternalInput").ap()
    eb_d = nc.dram_tensor("emB", [W, 1], f32, kind="ExternalInput").ap()
    out_d = nc.dram_tensor("out", [DIM, SLAB, W], f16, kind="ExternalOutput").ap()

    with nc.allow_low_precision(reason="2e-2 gate; fp16 has ~100x margin"), \
            TileContext(nc) as tc:
        with (
            tc.tile_pool(name="consts", bufs=1) as cpool,
            tc.tile_pool(name="xsp", bufs=3) as xsp,
            tc.tile_pool(name="ysp", bufs=2) as ysp,
            tc.tile_pool(name="kerp", bufs=2) as kerp,
            tc.tile_pool(name="kt1p", bufs=2) as kt1p,
            tc.tile_pool(name="ktp", bufs=1) as ktp,
            tc.tile_pool(name="accp", bufs=1) as accp,
            tc.tile_pool(name="stage", bufs=3) as stp,
            tc.tile_pool(name="gstage", bufs=2) as gstp,
        ):
            w1s_sb = cpool.tile([128, 3, DIM], f16)
            nc.sync.dma_start(out=w1s_sb, in_=w1s_d)
            w1r_sb = cpool.tile([128, 3, DIM], f16)
            nc.sync.dma_start(out=w1r_sb, in_=w1r_d)
            w2s_sb = cpool.tile([128, 3, 9], f16)
            nc.sync.dma_start(out=w2s_sb, in_=w2s_d)
            w2r_sb = cpool.tile([128, 3, 9], f16)
            nc.sync.dma_start(out=w2r_sb, in_=w2r_d)
            b1_sb = cpool.tile([DIM, 1], f32)
            nc.sync.dma_start(out=b1_sb, in_=b1_d)
            b2_sb = cpool.tile([9, 1], f32)
            nc.sync.dma_start(out=b2_sb, in_=b2_d)
            ym_sb = cpool.tile([DIM, 2], f32)
            nc.sync.dma_start(out=ym_sb, in_=ym_d)
            em_sb = cpool.tile([W, 1], f32)
            nc.sync.dma_start(out=em_sb, in_=em_d)
            ef_sb = cpool.tile([W, 1], f32)
            nc.sync.dma_start(out=ef_sb, in_=ef_d)
            ea_sb = cpool.tile([W, 1], f32)
            nc.sync.dma_start(out=ea_sb, in_=ea_d)
            eb_sb = cpool.tile([W, 1], f32)
            nc.sync.dma_start(out=eb_sb, in_=eb_d)

            # whole filter input; frame f is xt_sb[:, f]
            xt_sb = cpool.tile([W, T, DIM, FR], f16)
            nc.sync.dma_start(out=xt_sb, in_=xt_d)

            # per-pixel kernels, pixel-partitioned: kt[p, 9f+3di+dj, r]
            kt = ktp.tile([W, 48, SLAB], f16)
            ktr = kt[:, 0:45, :].rearrange(
                "p (t di dj) r -> p t di dj r", t=T, di=3, dj=3)
            kt_p1 = ktp.tile([W, 48, SLAB], f16)   # kt_p1[q] = kt[q+1]
            kt_m1 = ktp.tile([W, 48, SLAB], f16)   # kt_m1[q] = kt[q-1]
            nc.vector.memset(kt_p1[96:128], 0.0)
            nc.vector.memset(kt_m1[0:32], 0.0)

            # dj-separated accumulators (fp16), gacc for the gpsimd taps
            accs = []
            for dj in range(3):
                a = accp.tile([W, DIM, SLAB], f16, name=f"acc{dj}")
                nc.vector.memset(a, 0.0)
                accs.append(a)
            gacc = accp.tile([W, DIM, SLAB], f16)
            nc.gpsimd.memset(gacc, 0.0)

            xs_f_tiles = {}

            def load_xs(f):
                t = xsp.tile([128, GH, GW], f16, tag="xs")
                nc.sync.dma_start(out=t, in_=xs_d[:, f])
                xs_f_tiles[f] = t

            ys_tiles = {}
            ker_tiles = {}

            with (
                tc.tile_pool(name="ps1", bufs=4, space="PSUM") as ps1p,
                tc.tile_pool(name="ps2", bufs=4, space="PSUM") as ps2p,
            ):
                def conv1(f):
                    xs_f = xs_f_tiles[f]
                    ys = ysp.tile([128, GH, GW], f16, tag="ys")
                    ys_tiles[f] = ys
                    # conv2 reads cols dj..dj+128 of rows 1..34; zero pad cols
                    nc.vector.memset(ys[0:64, 1:35, 0:1], 0.0)
                    nc.vector.memset(ys[0:64, 1:35, 129:130], 0.0)
                    for rc in range(9):
                        g0 = 1 + 4 * rc
                        nr = 4 if rc < 8 else 2
                        ps = ps1p.tile([DIM, 4, W], f32, tag="ps1")
                        for dj in range(3):
                            nc.tensor.matmul(
                                ps[:, :nr, :],
                                lhsT=w1s_sb[:, dj, :],
                                rhs=xs_f[:, g0 - 1:g0 - 1 + nr, dj:dj + W],
                                start=(dj == 0), stop=False)
                        for dj in range(3):
                            nc.tensor.matmul(
                                ps[:, :nr, :],
                                lhsT=w1r_sb[64:128, dj, :],
                                rhs=xs_f[64:128, g0:g0 + nr, dj:dj + W],
                                start=False, stop=(dj == 2))
                        nc.scalar.activation(
                            ys[0:64, g0:g0 + nr, 1:129], ps[:, :nr],
                            Act.Prelu, bias=b1_sb, scale=1.0, alpha=0.2)
                    # zero y halo rows outside the image (conv2 zero-pad)
                    nc.scalar.activation(ys[0:64, 1:2, 1:129],
                                         ys[0:64, 1:2, 1:129],
                                         Act.Copy, scale=ym_sb[:, 0:1])
                    nc.scalar.activation(ys[0:64, 34:35, 1:129],
                                         ys[0:64, 34:35, 1:129],
                                         Act.Copy, scale=ym_sb[:, 1:2])
                    # stacked row-shifted copy: ys[64+c, g] = ys[c, g+1]
                    nc.sync.dma_start(out=ys[64:128, 1:34, :],
                                      in_=ys[0:64, 2:35, :])

                def conv2(f):
                    ys = ys_tiles[f]
                    ker_f = kerp.tile([16, SLAB, W], f16, tag="ker")
                    ker_tiles[f] = ker_f
                    for rc in range(8):
                        g0 = 2 + 4 * rc
                        ps2 = ps2p.tile([9, 4, W], f32, tag="ps2")
                        for dj in range(3):
                            nc.tensor.matmul(
                                ps2,
                                lhsT=w2s_sb[:, dj, :],
                                rhs=ys[:, g0 - 1:g0 + 3, dj:dj + W],
                                start=(dj == 0), stop=False)
                        for dj in range(3):
                            nc.tensor.matmul(
                                ps2,
                                lhsT=w2r_sb[64:128, dj, :],
                                rhs=ys[64:128, g0:g0 + 4, dj:dj + W],
                                start=False, stop=(dj == 2))
                        nc.scalar.activation(
                            ker_f[0:9, 4 * rc:4 * rc + 4, :], ps2,
                            Act.Identity, bias=b2_sb, scale=1.0)

                def post(f):
                    # pivot ker (9, r, w) -> (w, r, 9) via the XBAR
                    ker_f = ker_tiles[f]
                    kt1 = kt1p.tile([W, SLAB, 16], f16, tag="kt1")
                    nc.sync.dma_start(out=kt1, in_=ker_f, transpose=True)
                    # repack to (w, 9, r): taps outer, rows contiguous
                    nc.scalar.copy(
                        kt[:, 9 * f:9 * f + 9, :],
                        kt1[:, :, 0:9].rearrange("p r k -> p k r"))
                    # fold W-edge replicate-pad terms into the dj=1 slot
                    nc.vector.tensor_tensor(ktr[0:1, f, :, 1, :],
                                            ktr[0:1, f, :, 1, :],
                                            ktr[0:1, f, :, 0, :], Alu.add)
                    nc.vector.scalar_tensor_tensor(
                        out=ktr[96:128, f, :, 1, :],
                        in0=ktr[96:128, f, :, 2, :], scalar=em_sb[96:128, :],
                        in1=ktr[96:128, f, :, 1, :],
                        op0=Alu.mult, op1=Alu.add)
                    # partition-shifted kernel copies for dj=0 / dj=2 taps
                    nc.sync.dma_start(out=kt_p1[0:127, 9 * f:9 * f + 9, :],
                                      in_=kt[1:128, 9 * f:9 * f + 9, :])
                    nc.sync.dma_start(out=kt_m1[1:128, 9 * f:9 * f + 9, :],
                                      in_=kt[0:127, 9 * f:9 * f + 9, :])
                    # dynamic filtering for this frame
                    ksrc = [kt_p1, kt, kt_m1]
                    for di in range(3):
                        for dj in range(3):
                            kb = ksrc[dj][:, 9 * f + 3 * di + dj, :]\
                                .unsqueeze(1).broadcast_to((W, DIM, SLAB))
                            xt_sl = xt_sb[:, f, :, di:di + SLAB]
                            if (di, dj) in GP_TAPS:
                                prod = gstp.tile([W, DIM, SLAB], f16,
                                                 tag="gprod")
                                nc.gpsimd.tensor_tensor(prod, xt_sl, kb,
                                                        Alu.mult)
                                nc.gpsimd.tensor_tensor(gacc, gacc, prod,
                                                        Alu.add)
                            else:
                                prod = stp.tile([W, DIM, SLAB], f16,
                                                tag="prod")
                                nc.vector.tensor_tensor(prod, xt_sl, kb,
                                                        Alu.mult)
                                nc.vector.tensor_tensor(accs[dj], accs[dj],
                                                        prod, Alu.add)

                # frame schedule: keep the PE matmul stream dense by putting
                # conv1(f+1) between conv1(f) and conv2(f)
                load_xs(0)
                load_xs(1)
                conv1(0)
                load_xs(2)
                conv1(1)
                conv2(0)
                post(0)
                load_xs(3)
                conv1(2)
                conv2(1)
                post(1)
                load_xs(4)
                conv1(3)
                conv2(2)
                post(2)
                conv1(4)
                conv2(3)
                post(3)
                conv2(4)
                post(4)

            # u = sum_f xt_f  (for the normalization term c * S)
            u_sb = accp.tile([W, DIM, FR], f16)
            nc.vector.tensor_tensor(u_sb, xt_sb[:, 0], xt_sb[:, 1], Alu.add)
            for f in range(2, T):
                nc.vector.tensor_tensor(u_sb, u_sb, xt_sb[:, f], Alu.add)

            # normalization: out += c * S with c = 1/45 - mean(ker);
            # sum45 reads the folded kernel, undo the edge double-count
            sum45 = ktp.tile([W, SLAB], f16)
            kt_v = kt[:, 0:45, :].rearrange("p (t n) r -> p r t n", t=T)
            nc.vector.tensor_reduce(sum45, kt_v, axis=mybir.AxisListType.XY,
                                    op=Alu.add)
            c_sb = ktp.tile([W, SLAB], f16)
            nc.vector.tensor_scalar(c_sb, sum45, -1.0 / 45.0, 1.0 / 45.0,
                                    Alu.mult, Alu.add)
            corr = ktp.tile([W, SLAB], f16)
            kt_e = kt[:, 0:45, :].rearrange(
                "p (t di dj) r -> p r t di dj", t=T, di=3, dj=3)
            nc.vector.tensor_reduce(corr[0:32], kt_e[0:32, :, :, :, 0],
                                    axis=mybir.AxisListType.XY, op=Alu.add)
            nc.vector.tensor_reduce(corr[96:128], kt_e[96:128, :, :, :, 2],
                                    axis=mybir.AxisListType.XY, op=Alu.add)
            nc.vector.scalar_tensor_tensor(out=c_sb[0:32], in0=corr[0:32],
                                           scalar=ea_sb[0:32], in1=c_sb[0:32],
                                           op0=Alu.mult, op1=Alu.add)
            nc.vector.scalar_tensor_tensor(out=c_sb[96:128], in0=corr[96:128],
                                           scalar=eb_sb[96:128],
                                           in1=c_sb[96:128],
                                           op0=Alu.mult, op1=Alu.add)

            # S = 3-row vertical box of u (edge rows already clamped in xt)
            s_sb = accp.tile([W, DIM, SLAB], f16)
            nc.vector.tensor_tensor(s_sb, u_sb[:, :, 0:SLAB],
                                    u_sb[:, :, 1:SLAB + 1], Alu.add)
            nc.vector.tensor_tensor(s_sb, s_sb, u_sb[:, :, 2:SLAB + 2],
                                    Alu.add)

            # shifted + edge-doubled variants of c
            c_p1 = ktp.tile([W, SLAB], f16)
            c_m1 = ktp.tile([W, SLAB], f16)
            nc.vector.memset(c_p1[96:128], 0.0)
            nc.vector.memset(c_m1[0:32], 0.0)
            nc.sync.dma_start(out=c_p1[0:127], in_=c_sb[1:128])
            nc.sync.dma_start(out=c_m1[1:128], in_=c_sb[0:127])
            c_c = ktp.tile([W, SLAB], f16)
            nc.vector.tensor_scalar(c_c, c_sb, ef_sb, None, Alu.mult)
            for dj, csrc in ((0, c_p1), (1, c_c), (2, c_m1)):
                cb = csrc.unsqueeze(1).broadcast_to((W, DIM, SLAB))
                prod = stp.tile([W, DIM, SLAB], f16, tag="prod")
                nc.vector.tensor_tensor(prod, s_sb, cb, Alu.mult)
                nc.vector.tensor_tensor(accs[dj], accs[dj], prod, Alu.add)
            nc.vector.tensor_tensor(accs[1], accs[1], gacc, Alu.add)

            # merge: out[w] = acc1[w] + acc0[w-1] + acc2[w+1] via partition
            # shifts (DMA), then one XBAR transpose of [w, (c r)]
            a0s = accp.tile([W, DIM, SLAB], f16)
            a2s = accp.tile([W, DIM, SLAB], f16)
            nc.vector.memset(a0s[0:32], 0.0)
            nc.vector.memset(a2s[96:128], 0.0)
            nc.sync.dma_start(out=a0s[1:128], in_=accs[0][0:127])
            nc.sync.dma_start(out=a2s[0:127], in_=accs[2][1:128])
            macc = accp.tile([W, DIM, SLAB], f16)
            nc.vector.tensor_tensor(macc, accs[1], a0s, Alu.add)
            nc.vector.tensor_tensor(macc, macc, a2s, Alu.add)

            obig = accp.tile([128, 16, 128], f16)
            nc.sync.dma_start(
                out=obig,
                in_=macc.rearrange("p (o a) r -> p o (a r)", o=16, a=4),
                transpose=True)
            # obig[m, o, w] = macc[w, 128o + m]; c = 4o + m//32, r = m%32
            out_v = out_d.rearrange("(o c4) r w -> (c4 r) o w", o=16, c4=4)
            nc.sync.dma_start(out=out_v, in_=obig)

    return nc


def _get_program():
    if "nc" not in _PROGRAM_CACHE:
        nc = _build_program()
        nc.finalize()
        _PROGRAM_CACHE["nc"] = nc
    return _PROGRAM_CACHE["nc"]


def _host_prep(x, w1, b1, w2, b2):
    """Build the 8 per-core input maps from full inputs."""
    x = np.asarray(x, dtype=np.float32)
    w1 = np.asarray(w1, dtype=np.float32)
    b1 = np.asarray(b1, dtype=np.float32)
    w2 = np.asarray(w2, dtype=np.float32)
    b2 = np.asarray(b2, dtype=np.float32)

    # stacked conv weights: pairs di=a in partitions 64a..64a+63, leftover
    # di=2 in partitions 64..127 of the r-variant
    w1s = np.zeros((128, 3, DIM), dtype=np.float16)
    w1r = np.zeros((128, 3, DIM), dtype=np.float16)
    for a in range(2):
        # w1s[c + 64a, dj, o] = w1[o, c, a, dj]
        w1s[64 * a:64 * a + 64] = w1[:, :, a, :].transpose(1, 2, 0)
    w1r[64:128] = w1[:, :, 2, :].transpose(1, 2, 0)
    w2s = np.zeros((128, 3, 9), dtype=np.float16)
    w2r = np.zeros((128, 3, 9), dtype=np.float16)
    for a in range(2):
        w2s[64 * a:64 * a + 64] = w2[:, :, a, :].transpose(1, 2, 0)
    w2r[64:128] = w2[:, :, 2, :].transpose(1, 2, 0)

    b1c = np.ascontiguousarray(b1.reshape(DIM, 1))
    b2c = np.ascontiguousarray(b2.reshape(9, 1))
    emask = np.zeros((W, 1), dtype=np.float32)
    emask[127, 0] = 1.0
    efold = np.ones((W, 1), dtype=np.float32)
    efold[0, 0] = 2.0
    efold[127, 0] = 2.0
    emA = np.zeros((W, 1), dtype=np.float32)
    emA[0, 0] = 1.0 / 45.0
    emB = np.zeros((W, 1), dtype=np.float32)
    emB[127, 0] = 1.0 / 45.0

    x16 = x.astype(np.float16)
    in_maps = []
    for core in range(NCORES):
        b, s = divmod(core, 4)
        r0 = s * SLAB
        # conv input, stacked: xs[c+64a, f, g, w] = xpad[c, f, r0-2+g+a, w-1]
        xs = np.zeros((128, T, GH, GW), dtype=np.float16)
        for a in range(2):
            lo = r0 - 2 + a
            hi = lo + GH            # rows lo .. hi-1
            clo = max(0, lo)
            chi = min(H, hi)
            if chi > clo:
                xs[64 * a:64 * a + 64, :, clo - lo:chi - lo, 1:129] = \
                    x16[b, :, :, clo:chi, :]
        # filter input, pixel-partitioned: xt[w, f, c, r]
        rows = np.clip(np.arange(r0 - 1, r0 + 33), 0, H - 1)
        xt = np.ascontiguousarray(
            x16[b][:, :, rows, :].transpose(3, 1, 0, 2))
        ymask = np.ones((DIM, 2), dtype=np.float32)
        if s == 0:
            ymask[:, 0] = 0.0
        if s == 3:
            ymask[:, 1] = 0.0
        in_maps.append({
            "xs": xs, "xt": xt, "w1s": w1s, "w1r": w1r, "w2s": w2s,
            "w2r": w2r, "b1c": b1c, "b2c": b2c, "ymask": ymask,
            "emask": emask, "efold": efold, "emA": emA, "emB": emB,
        })
    return in_maps


def kernel(x, w1, b1, w2, b2):
    from concourse.bass_utils import run_bass_kernel_spmd

    nc = _get_program()
    in_maps = _host_prep(x, w1, b1, w2, b2)
    res = run_bass_kernel_spmd(nc, in_maps, list(range(NCORES)))
    out = np.zeros((2, DIM, H, W), dtype=np.float32)
    for core in range(NCORES):
        b, s = divmod(core, 4)
        out[b, :, s * SLAB:(s + 1) * SLAB, :] = \
            res.results[core]["out"].astype(np.float32)
    return out
